# revision 5
# baseline (speedup 1.0000x reference)
"""AmplitudeWeightedPhaseAttention Trainium2 kernel (8 NeuronCores, SPMD).

Math: the reference's [B,Sq,Sk,F] tensor collapses algebraically.
With rfft bin features re/im and amp2 = re^2 + im^2:
    t  = amp2^(-1/4)       u = re*t   v = im*t   w = amp2^(+1/4)
    num[i,j] = sum_f u_q u_k + v_q v_k        (rank-128 matmul: v==0 at f=0,64)
    den[i,j] = sum_f w_q w_k                  (rank-65 matmul)
    weights  = softmax_j(num/den + 1)         out = weights @ V
Sharding: core c owns batch c//4, query rows (c%4)*256..+256.  Each core
computes its [256, 1024] score block in natural [i_p, j] layout (softmax
normalization is then a per-partition scalar), transposes the normalized
weights via PE to feed the P@V matmul, and writes its weights block and
(transposed) output block.  No collectives.
"""

import numpy as np
from contextlib import ExitStack

B, S, H = 2, 1024, 128
F = H // 2 + 1  # 65
NCORES = 8
QBLK = S // 4  # 256 query rows per core
NJT = S // 128  # 8 key tiles
NIT = QBLK // 128  # 2 query sub-tiles

WEIGHTS_BF16 = True  # store weights output as bf16, upcast on host
ET_VIA_DMA_XBAR = False  # transpose normalized weights via DMA xbar vs PE

_CACHE = {}


def _dft_consts():
    h = np.arange(H, dtype=np.float64)[:, None]
    f = np.arange(F, dtype=np.float64)[None, :]
    C = np.cos(2 * np.pi * h * f / H)
    Sn = -np.sin(2 * np.pi * h * f / H)
    Sn[:, 0] = 0.0
    Sn[:, F - 1] = 0.0  # exactly zero at DC and Nyquist
    return np.concatenate([C, Sn], axis=1).astype(np.float32)  # [128, 130]


def _build():
    import concourse.bass as bass
    import concourse.tile as tile
    from concourse import bacc, mybir

    f32 = mybir.dt.float32
    bf16 = mybir.dt.bfloat16
    AF = mybir.ActivationFunctionType

    nc = bacc.Bacc("TRN2", target_bir_lowering=False, debug=False,
                   num_devices=NCORES)
    Qs = nc.dram_tensor("Qs", [QBLK, H], f32, kind="ExternalInput").ap()
    K = nc.dram_tensor("K", [S, H], f32, kind="ExternalInput").ap()
    V = nc.dram_tensor("V", [S, H], f32, kind="ExternalInput").ap()
    CD = nc.inline_tensor(_dft_consts(), "CDconst").ap()  # [128, 130]
    IDT = nc.inline_tensor(np.eye(H, dtype=np.float32), "IDTconst").ap()
    ow_dt = bf16 if WEIGHTS_BF16 else f32
    OW = nc.dram_tensor("OW", [QBLK, S], ow_dt, kind="ExternalOutput").ap()
    OO = nc.dram_tensor("OO", [H, QBLK], f32, kind="ExternalOutput").ap()

    with ExitStack() as ctx:
        tc = ctx.enter_context(tile.TileContext(nc))
        consts = ctx.enter_context(tc.tile_pool(name="consts", bufs=1))
        big = ctx.enter_context(tc.tile_pool(name="big", bufs=1))
        ftmp = ctx.enter_context(tc.tile_pool(name="ftmp", bufs=1))
        ps = ctx.enter_context(tc.tile_pool(name="ps", bufs=4, space="PSUM"))

        # --- ACT table preload: first ACT op is Ln so the loaded set
        # (natural_log_exp_and_others) covers Ln/Exp/Square/Copy. ---
        junk = consts.tile([128, 1], f32)
        nc.vector.memset(junk[:], 1.0)
        junk2 = consts.tile([128, 1], f32)
        nc.scalar.activation(junk2[:], junk[:], AF.Ln)

        # --- DMA inputs ---
        idt = consts.tile([H, H], f32)
        nc.sync.dma_start(out=idt[:], in_=IDT[:])
        cdf = consts.tile([H, 2 * F], f32)
        nc.sync.dma_start(out=cdf[:], in_=CD[:])
        cdb = consts.tile([H, 2 * F], bf16)
        nc.vector.tensor_copy(cdb[:], cdf[:])

        kn = big.tile([128, NJT, H], f32)
        nc.sync.dma_start(out=kn[:], in_=K.rearrange("(t p) h -> p t h", p=128))
        qn = big.tile([128, NIT, H], f32)
        nc.sync.dma_start(out=qn[:], in_=Qs.rearrange("(t p) h -> p t h", p=128))
        vn = big.tile([128, NJT, H], f32)
        nc.sync.dma_start(out=vn[:], in_=V.rearrange("(t p) h -> p t h", p=128))
        vb = big.tile([128, NJT, H], bf16)
        nc.vector.tensor_copy(vb[:], vn[:])

        # --- transposes: K^T, Q^T (h on partitions) via PE ---
        kt_ps = ps.tile([128, S], f32, tag="ps")
        for t in range(NJT):
            nc.tensor.transpose(kt_ps[:, t * 128:(t + 1) * 128], kn[:, t, :], idt[:])
        ktb = big.tile([128, S], bf16)
        nc.scalar.copy(ktb[:], kt_ps[:])

        qt_ps = ps.tile([128, QBLK], f32, tag="ps")
        for t in range(NIT):
            nc.tensor.transpose(qt_ps[:, t * 128:(t + 1) * 128], qn[:, t, :], idt[:])
        qtb = big.tile([128, QBLK], bf16)
        nc.vector.tensor_copy(qtb[:], qt_ps[:])

        # --- DFT + features for X in {K (N=1024), Q (N=256)} ---
        # Two ACT table phases: {Square, Ln} first (natural_log set), then
        # all {Exp} (exp_and_others set) — exactly two table loads.
        def dft_ln_phase(xtb, N, qk):
            re_ps = ps.tile([F, N], f32, tag="ps")
            im_ps = ps.tile([F, N], f32, tag="ps")
            for c0 in range(0, N, 512):
                c1 = min(c0 + 512, N)
                nc.tensor.matmul(re_ps[:, c0:c1], cdb[:, 0:F], xtb[:, c0:c1],
                                 start=True, stop=True)
                nc.tensor.matmul(im_ps[:, c0:c1], cdb[:, F:2 * F], xtb[:, c0:c1],
                                 start=True, stop=True)
            sqre = ftmp.tile([F, N], bf16, tag=f"sqre{qk}")
            nc.scalar.activation(sqre[:], re_ps[:], AF.Square)
            sqim = ftmp.tile([F, N], bf16, tag=f"sqim{qk}")
            nc.scalar.activation(sqim[:], im_ps[:], AF.Square)
            amp2 = ftmp.tile([F, N], bf16, tag=f"amp2{qk}")
            nc.vector.tensor_add(amp2[:], sqre[:], sqim[:])
            lg = ftmp.tile([F, N], f32, tag=f"lg{qk}")
            nc.scalar.activation(lg[:], amp2[:], AF.Ln)
            return re_ps, im_ps, lg

        def exp_phase(re_ps, im_ps, lg, N, Ux, Vx, Wx, qk):
            tq = ftmp.tile([F, N], f32, tag=f"tq{qk}")
            nc.scalar.activation(tq[:], lg[:], AF.Exp, scale=-0.25)
            nc.scalar.activation(Wx[:], lg[:], AF.Exp, scale=0.25)
            nc.vector.tensor_mul(Ux[:], re_ps[:], tq[:])
            # v(f)=im(f)*t(f) for f=0..63; v(0)==0 since im(0)==0
            nc.vector.tensor_mul(Vx[:], im_ps[0:F - 1, :], tq[0:F - 1, :])

        uk = big.tile([F, S], bf16)
        vk = big.tile([F - 1, S], bf16)
        wk = big.tile([F, S], bf16)
        uq = big.tile([F, QBLK], bf16)
        vq = big.tile([F - 1, QBLK], bf16)
        wq = big.tile([F, QBLK], bf16)
        rek, imk, lgk = dft_ln_phase(ktb, S, "k")
        req, imq, lgq = dft_ln_phase(qtb, QBLK, "q")
        exp_phase(rek, imk, lgk, S, uk, vk, wk, "k")
        exp_phase(req, imq, lgq, QBLK, uq, vq, wq, "q")

        # --- scores + softmax per query sub-tile (natural [i_p, j] layout) ---
        wb_tiles = []
        for it in range(NIT):
            i0, i1 = it * 128, (it + 1) * 128
            num_ps = ps.tile([128, S], f32, tag="ps")
            den_ps = ps.tile([128, S], f32, tag="ps")
            for c0 in range(0, S, 512):
                c1 = c0 + 512
                nc.tensor.matmul(num_ps[:, c0:c1], uq[:, i0:i1], uk[:, c0:c1],
                                 start=True, stop=False)
                nc.tensor.matmul(num_ps[:, c0:c1], vq[:, i0:i1], vk[:, c0:c1],
                                 start=False, stop=True)
                nc.tensor.matmul(den_ps[:, c0:c1], wq[:, i0:i1], wk[:, c0:c1],
                                 start=True, stop=True)
            inv = ftmp.tile([128, S], f32, tag="inv")
            nc.vector.reciprocal_approx_fast(out=inv[:], in_=den_ps[:])
            pa = ftmp.tile([128, S], f32, tag="pa")
            nc.vector.tensor_mul(pa[:], num_ps[:], inv[:])
            e = ftmp.tile([128, S], bf16, tag="e")
            sumexp = ftmp.tile([128, 1], f32, tag="sumexp")
            nc.scalar.activation(e[:], pa[:], AF.Exp, bias=1.0,
                                 accum_out=sumexp[:])
            r = ftmp.tile([128, 1], f32, tag="r")
            nc.vector.reciprocal(r[:], sumexp[:])
            # normalized weights: bf16 copy for P@V + the output row block
            wb = big.tile([128, S], bf16, tag=f"wb{it}")
            nc.vector.tensor_scalar_mul(wb[:], e[:], r[:])
            wb_tiles.append(wb)
            if WEIGHTS_BF16:
                nc.sync.dma_start(out=OW[i0:i1, :], in_=wb[:])
            else:
                oww = ftmp.tile([128, S], f32, tag="oww")
                nc.vector.tensor_scalar_mul(oww[:], e[:], r[:])
                nc.sync.dma_start(out=OW[i0:i1, :], in_=oww[:])

        # --- transpose normalized weights to [j_p, i] for P@V ---
        et_tiles = []
        if ET_VIA_DMA_XBAR:
            for jt in range(NJT):
                et = big.tile([128, QBLK], bf16, tag=f"et{jt}")
                for it in range(NIT):
                    nc.sync.dma_start_transpose(
                        out=et[:, it * 128:(it + 1) * 128],
                        in_=wb_tiles[it][:, jt * 128:(jt + 1) * 128])
                et_tiles.append(et)
        else:
            idtb = consts.tile([H, H], bf16)
            nc.vector.tensor_copy(idtb[:], idt[:])
            for g in range(2):  # two groups of 4 j-tiles share one psum tile
                wt_ps = ps.tile([128, 4 * QBLK], bf16, tag="ps")
                for lj in range(4):
                    jt = g * 4 + lj
                    for it in range(NIT):
                        nc.tensor.transpose(
                            wt_ps[:, lj * QBLK + it * 128:lj * QBLK + (it + 1) * 128],
                            wb_tiles[it][:, jt * 128:(jt + 1) * 128], idtb[:])
                for lj in range(4):
                    jt = g * 4 + lj
                    et = big.tile([128, QBLK], bf16, tag=f"et{jt}")
                    nc.scalar.copy(et[:], wt_ps[:, lj * QBLK:(lj + 1) * QBLK])
                    et_tiles.append(et)

        # --- P@V: out^T[h, i] accumulated over j tiles ---
        av_ps = ps.tile([128, QBLK], f32, tag="ps")
        for jt in range(NJT):
            nc.tensor.matmul(av_ps[:], vb[:, jt, :], et_tiles[jt][:],
                             start=(jt == 0), stop=(jt == NJT - 1))
        oo = big.tile([H, QBLK], f32)
        nc.vector.tensor_copy(oo[:], av_ps[:])
        nc.sync.dma_start(out=OO[:], in_=oo[:])

    nc.compile()
    return nc


def _get_nc():
    if "nc" not in _CACHE:
        _CACHE["nc"] = _build()
    return _CACHE["nc"]


def kernel(Q, K, V):
    from concourse.bass_utils import run_bass_kernel_spmd

    Q = np.ascontiguousarray(np.asarray(Q, dtype=np.float32))
    K = np.ascontiguousarray(np.asarray(K, dtype=np.float32))
    V = np.ascontiguousarray(np.asarray(V, dtype=np.float32))
    nc = _get_nc()
    in_maps = []
    for c in range(NCORES):
        b, qb = c // 4, c % 4
        in_maps.append({
            "Qs": np.ascontiguousarray(Q[b, qb * QBLK:(qb + 1) * QBLK]),
            "K": K[b],
            "V": V[b],
        })
    res = run_bass_kernel_spmd(nc, in_maps, core_ids=list(range(NCORES)))
    output = np.empty((B, S, H), np.float32)
    weights = np.empty((B, S, S), np.float32)
    for c in range(NCORES):
        b, qb = c // 4, c % 4
        rr = res.results[c]
        weights[b, qb * QBLK:(qb + 1) * QBLK, :] = rr["OW"].astype(np.float32)
        output[b, qb * QBLK:(qb + 1) * QBLK, :] = rr["OO"].T
    return output, weights


if __name__ == "__main__":
    rng = np.random.default_rng(0)
    Q = rng.standard_normal((B, S, H)).astype(np.float32)
    K = rng.standard_normal((B, S, H)).astype(np.float32)
    V = rng.standard_normal((B, S, H)).astype(np.float32)
    out, w = kernel(Q, K, V)
    print("kernel ran:", out.shape, w.shape)
